# revision 28
# baseline (speedup 1.0000x reference)
"""Trainium2 Bass kernel for nn_EulerCausalAttention_75892072121064.

Sharding: batch*heads across 8 cores (core c -> batch c//4, heads 4*(c%4)..+4).
Each core computes transposed-layout causal attention for its (b, 4-head)
slice plus the out-proj partial, writing outT (D, S) in fp16; the host sums
the 4 per-batch partials and transposes back.

v2 design (vs the LUT-exact baseline):
- The reference's 4096-entry LUT quantizes theta to ~7.7e-4 rad; computing
  continuous sin/cos instead lands ~7e-4 rel err vs the 2e-2 gate, so the
  whole round/wrap DVE chain is replaced by host-side prep: the host ships
  wrapped angles (x*inv + b [+ pi/2 for cos rows], wrapped to [-pi, pi]) as
  fp16 [128, S] per (head, q/k), and the device runs ONE big Sin per tile
  (ACT cost is free-size only: 8 Sins of [128,2048] ~= 15us vs 64 of
  [64,512] ~= 46us).
- All matmul operands fp16 (full PE rate, 2x DVE ops, half the DMA).
  exp(score/sqrt(128) - 5) keeps attn weights in fp16 range (max score ~=
  125 on the q==k diagonal); the -5 cancels in normalization.
- Diagonal score blocks are column-trimmed: matmul/exp/attnV only touch the
  causally valid [r*128:512] window (saves ~12k PE columns + ~10us ACT).
- V-proj groups are interleaved with attention qw windows (group g feeds
  exactly the vt tiles window qw=g needs), so PE ramps while DMA streams.
- PSUM: scores+vproj share a 4-bank ring; attnV 2; outproj 2 = 8 banks.
"""
import sys

import numpy as np

sys.path.insert(0, "/opt/trn_rl_repo")

from concourse import bacc, mybir  # noqa: E402
import concourse.tile as tile  # noqa: E402
from concourse.bass_utils import run_bass_kernel_spmd  # noqa: E402

B, S, D, H, DH = 2, 2048, 1024, 16, 64
LUT = 4096
TWO_PI = 2.0 * np.pi
SCALE = float(np.sqrt(np.float32(2.0 * DH)))  # sqrt(128) as f32
NCORES = 8
HPC = 4            # heads per core
CW = HPC * DH      # 256 cols per core
SQW = 512          # q window
NQW = S // SQW
KBS = 128          # k block size
NS = S // 128      # seq tiles
EB = -5.0          # exp bias: keeps fp16 attn weights in range, cancels in norm

F32 = mybir.dt.float32
F16 = mybir.dt.float16
AF = mybir.ActivationFunctionType
ALU = mybir.AluOpType

_CACHE = {}


def _seed_const(nc, val):
    key = (F32, float(val))
    if key in nc.const_aps.aps:
        return
    t = nc.alloc_sbuf_tensor(f"const-f32-{val}", [128, 1], F32)
    nc.gpsimd.memset(t.ap(), float(val))
    nc.const_aps.aps[key] = t.ap()


def _build_nc():
    nc = bacc.Bacc("TRN2", debug=False, num_devices=NCORES)
    _seed_const(nc, 0.0)
    _seed_const(nc, EB)
    nc.all_engine_barrier()

    # thA = [q0|k0|q1|k1], thB = [q2|k2|q3|k3] along the free dim
    thA_d = nc.dram_tensor("thA", [128, 4 * S], F16, kind="ExternalInput")
    thB_d = nc.dram_tensor("thB", [128, 4 * S], F16, kind="ExternalInput")
    # x / vw shipped pre-swizzled so each lands in ONE DMA issue per tile:
    # xg_d[g][p, od*512+q] = x[b].T[od*128+p, g*512+q]
    xg_d = [nc.dram_tensor(f"xg{g}", [128, 8 * SQW], F16, kind="ExternalInput")
            for g in range(NQW)]
    vwb_d = nc.dram_tensor("vwb", [128, 8 * CW], F16, kind="ExternalInput")
    owT_d = nc.dram_tensor("owT", [CW, D], F16, kind="ExternalInput")
    tri_d = nc.dram_tensor("tri", [128, 128], F16, kind="ExternalInput")
    # outb[qw][p, od*512+q] = outT[od*128+p, qw*512+q]
    outb_d = [nc.dram_tensor(f"outb{qw}", [128, 8 * SQW], F16,
                             kind="ExternalOutput") for qw in range(NQW)]

    inv_scale = float(1.0 / np.float32(SCALE))

    with tile.TileContext(nc) as tc:
        with (
            tc.tile_pool(name="pp", bufs=1) as pp,
            tc.tile_pool(name="atp", bufs=2) as atp,
            tc.tile_pool(name="wk", bufs=2) as wk,
            tc.tile_pool(name="scp", bufs=2, space="PSUM") as scp,
            tc.tile_pool(name="opp", bufs=4, space="PSUM") as opp,
        ):
            # ---- DMA: vwb + thetas + x group 0 first, rest streamed ----
            vwb = pp.tile([128, 8 * CW], F16, tag="vwb")
            nc.sync.dma_start(vwb[:], vwb_d[:])
            xgb = [pp.tile([128, 8 * SQW], F16, tag=f"xg{g}", name=f"xg{g}")
                   for g in range(NQW)]
            qt, kt = [], []
            with tc.tile_pool(name="thp", bufs=2) as thp:
                th_src = [(thA_d, 0), (thA_d, 2 * S), (thB_d, 0),
                          (thB_d, 2 * S)]
                for h in range(HPC):
                    dram, off = th_src[h]
                    th = thp.tile([128, 2 * S], F16, tag="th", name=f"th{h}")
                    nc.sync.dma_start(th[:, 0:S], dram[:, off:off + S])
                    nc.sync.dma_start(th[:, S:2 * S],
                                      dram[:, off + S:off + 2 * S])
                    if h == 0:
                        nc.sync.dma_start(xgb[0][:], xg_d[0][:])
                    elif h == 1:
                        for g in range(1, NQW):
                            nc.sync.dma_start(xgb[g][:], xg_d[g][:])
                    elif h == 2:
                        tri_sb = pp.tile([128, 128], F16, tag="tri")
                        nc.sync.dma_start(tri_sb[:], tri_d[:])
                        ow2 = []
                        for hp in range(2):
                            t = pp.tile([128, D], F16, tag=f"ow{hp}",
                                        name=f"ow{hp}")
                            nc.sync.dma_start(
                                t[:], owT_d[hp * 128:(hp + 1) * 128, :])
                            ow2.append(t)
                    # one Sin per (head, q/k)
                    q = pp.tile([128, S], F16, tag=f"qt{h}", name=f"qt{h}")
                    nc.scalar.activation(q[:], th[:, 0:S], AF.Sin,
                                         bias=0.0, scale=1.0)
                    qt.append(q)
                    k = pp.tile([128, S], F16, tag=f"kt{h}", name=f"kt{h}")
                    nc.scalar.activation(k[:], th[:, S:2 * S], AF.Sin,
                                         bias=0.0, scale=1.0)
                    kt.append(k)

            vt = [pp.tile([128, HPC * 65], F16, tag=f"v{si}", name=f"v{si}")
                  for si in range(NS)]

            def vproj_group(g):
                for j in range(4):
                    si = 4 * g + j
                    vps = opp.tile([128, SQW], F32, tag="o", name="vps")
                    for od in range(8):
                        nc.tensor.matmul(
                            vps[:, 0:CW],
                            xgb[g][:, od * SQW + j * 128:
                                   od * SQW + (j + 1) * 128],
                            vwb[:, od * CW:(od + 1) * CW],
                            start=(od == 0), stop=(od == 7),
                        )
                    dst = vt[si][:].rearrange(
                        "p (h w) -> p h w", w=65)[:, :, 0:64]
                    src = vps[:, 0:CW].rearrange("p (h w) -> p h w", w=64)
                    nc.vector.tensor_copy(dst, src)
                    ones = vt[si][:].rearrange(
                        "p (h w) -> p h w", w=65)[:, :, 64:65]
                    nc.gpsimd.memset(ones, 1.0)

            vproj_group(0)
            vproj_group(1)

            # Attention, software-pipelined with a 2-head lag that CARRIES
            # ACROSS window boundaries: the attnV chain for a head whose exps
            # are long done is interleaved into the exp-paced score phase of
            # a later head (possibly of the next qw window), so the PE never
            # starves; norm/out-proj of window qw are emitted as soon as its
            # last attnV chain drains (mid window qw+1).
            states = {}
            pend = []          # (qw, h, kb) attnV work not yet emitted
            o_cur = {}         # (qw, h) -> psum tile of in-flight chain
            done_q = []        # qw values whose avs fully emitted

            def emit_avs(n):
                for _ in range(n):
                    if not pend:
                        return
                    q_, h2, kb = pend.pop(0)
                    st = states[q_]
                    nkb_ = st["nkb"]
                    if kb == 0:
                        o_cur[(q_, h2)] = opp.tile([128, SQW], F32, tag="o",
                                                   name="o_ps")
                    at, coff, c0 = st["atmap"][h2][kb]
                    nc.tensor.matmul(
                        o_cur[(q_, h2)][0:65, c0:SQW],
                        vt[kb][:, h2 * 65:(h2 + 1) * 65],
                        at[:, coff + c0:coff + SQW],
                        start=(kb == 0), stop=(kb == nkb_ - 1),
                        skip_group_check=True,
                    )
                    if kb == nkb_ - 1:
                        o_ps = o_cur.pop((q_, h2))
                        # ACT copy: proven partition-shift (psum p64 -> p0)
                        nc.scalar.copy(
                            st["srow4"][0:1, h2 * SQW:(h2 + 1) * SQW],
                            o_ps[64:65, :])
                        oraw = wk.tile([64, SQW], F16, tag=f"oraw{h2}",
                                       name=f"oraw{h2}", bufs=1)
                        nc.vector.tensor_copy(oraw[:], o_ps[0:64, :])
                        st["oraws"][h2] = oraw
                        if h2 == 3:
                            done_q.append(q_)

            def emit_norm_oproj():
                while done_q:
                    q_ = done_q.pop(0)
                    st = states.pop(q_)
                    srow4 = st["srow4"]
                    # in-place reciprocal of the 4 denominator rows
                    nc.vector.reciprocal_approx_fast(srow4[:], srow4[:])
                    sre16 = wk.tile([1, HPC * SQW], F16, tag="sre16",
                                    name="sre16")
                    nc.vector.tensor_copy(sre16[:], srow4[:])
                    pairs = [wk.tile([128, SQW], F16, tag=f"pair{hp}",
                                     name=f"pair{hp}") for hp in range(2)]
                    for h_ in range(HPC):
                        bc = wk.tile([64, SQW], F16, tag="bc", name="bc")
                        nc.gpsimd.partition_broadcast(
                            bc[:], sre16[0:1, h_ * SQW:(h_ + 1) * SQW])
                        rows = slice((h_ % 2) * 64, (h_ % 2) * 64 + 64)
                        nc.vector.tensor_tensor(pairs[h_ // 2][rows, :],
                                                st["oraws"][h_][:], bc[:],
                                                ALU.mult)
                    prq = wk.tile([128, 8 * SQW], F16, tag="prq", name="prq",
                                  bufs=1)
                    for od in range(8):
                        pr = opp.tile([128, SQW], F32, tag="o", name="pr")
                        nc.tensor.matmul(pr[:],
                                         ow2[0][:, od * 128:(od + 1) * 128],
                                         pairs[0][:], start=True, stop=False)
                        nc.tensor.matmul(pr[:],
                                         ow2[1][:, od * 128:(od + 1) * 128],
                                         pairs[1][:], start=False, stop=True)
                        nc.vector.tensor_copy(
                            prq[:, od * SQW:(od + 1) * SQW], pr[:])
                        if od == 3:
                            nc.sync.dma_start(outb_d[q_][:, 0:4 * SQW],
                                              prq[:, 0:4 * SQW])
                    nc.sync.dma_start(outb_d[q_][:, 4 * SQW:8 * SQW],
                                      prq[:, 4 * SQW:8 * SQW])

            for qw in range(NQW):
                nkb = 4 * qw + 4
                states[qw] = {
                    "nkb": nkb,
                    "srow4": wk.tile([1, HPC * SQW], F32, tag="srow4",
                                     name="srow4", bufs=2),
                    "atmap": [None] * HPC,
                    "oraws": [None] * HPC,
                }
                for h in range(HPC):
                    if h >= 2:
                        pend.extend((qw, h - 2, kb) for kb in range(nkb))
                    ats = {}
                    # off-diagonal kb pairs share one 2-bank psum supertile
                    # and ONE exp (consumers lag 2 heads behind)
                    for kb in range(0, 4 * qw, 2):
                        scs = scp.tile([128, 2 * SQW], F32, tag="sc",
                                       name="scs")
                        for half in range(2):
                            nc.tensor.matmul(
                                scs[:, half * SQW:(half + 1) * SQW],
                                kt[h][:, (kb + half) * KBS:
                                      (kb + half + 1) * KBS],
                                qt[h][:, qw * SQW:(qw + 1) * SQW],
                                start=True, stop=True,
                            )
                            emit_avs(1)
                            emit_norm_oproj()
                        at2 = atp.tile([128, 2 * SQW], F16, tag="at",
                                       name="at2", bufs=20)
                        nc.scalar.activation(at2[:], scs[:], AF.Exp,
                                             bias=EB, scale=inv_scale)
                        ats[kb] = (at2, 0, 0)
                        ats[kb + 1] = (at2, SQW, 0)
                    for r in range(4):
                        kb = 4 * qw + r
                        c0 = r * 128
                        scd = scp.tile([128, 2 * SQW], F32, tag="sc",
                                       name="scd")
                        nc.tensor.matmul(
                            scd[:, c0:SQW],
                            kt[h][:, kb * KBS:(kb + 1) * KBS],
                            qt[h][:, qw * SQW + c0:(qw + 1) * SQW],
                            start=True, stop=True,
                        )
                        atd = atp.tile([128, SQW], F16, tag="atd",
                                       name="atd", bufs=16)
                        nc.scalar.activation(atd[:, c0:SQW], scd[:, c0:SQW],
                                             AF.Exp, bias=EB, scale=inv_scale)
                        nc.vector.tensor_tensor(
                            atd[:, c0:c0 + 128], atd[:, c0:c0 + 128],
                            tri_sb[:], ALU.mult,
                        )
                        ats[kb] = (atd, 0, c0)
                        emit_avs(1)
                        emit_norm_oproj()
                    states[qw]["atmap"][h] = ats
                # queue the trailing 2 heads; they drain during qw+1 (or the
                # final epilogue below)
                pend.extend((qw, 2, kb) for kb in range(nkb))
                pend.extend((qw, 3, kb) for kb in range(nkb))

                # PE filler while the tail exps drain
                if qw < 2:
                    vproj_group(qw + 2)

            emit_avs(len(pend))
            emit_norm_oproj()

    nc.compile()
    return nc


def _prep_inputs(x, w_q, b_q, w_k, b_k, v_w, out_w):
    """Build the 8 per-core input maps (host-side sharding + angle prep)."""
    in_maps = []
    tri = np.triu(np.ones((128, 128), dtype=np.float16))  # valid: q >= k
    half_pi = np.pi / 2

    def wrap(v):
        return ((v + np.pi) % TWO_PI) - np.pi

    for c in range(NCORES):
        b = c // 4
        h0 = (c % 4) * HPC
        cols = np.arange(h0 * DH, h0 * DH + CW)
        im = {"tri": tri}
        xb = x[b].astype(np.float64)
        ths = []
        for hh in range(HPC):
            hg = h0 + hh
            for nm, w_, b_ in (("q", w_q, b_q), ("k", w_k, b_k)):
                th = xb[:, hg * DH:(hg + 1) * DH] / (
                    1.0 + np.abs(w_[hg].astype(np.float64))) + b_[hg]
                t = np.concatenate([wrap(th + half_pi).T, wrap(th).T], axis=0)
                ths.append(t.astype(np.float16))
        im["thA"] = np.ascontiguousarray(np.concatenate(ths[0:4], axis=1))
        im["thB"] = np.ascontiguousarray(np.concatenate(ths[4:8], axis=1))
        # xg{g}[p, od*512+q] = x[b].T[od*128+p, g*512+q]
        xt = x[b].T.astype(np.float16).reshape(8, 128, NQW, SQW)
        for g in range(NQW):
            im[f"xg{g}"] = np.ascontiguousarray(
                xt[:, :, g, :].transpose(1, 0, 2).reshape(128, 8 * SQW))
        # vwb[p, od*256+j] = v_w[cols].T[od*128+p, j]
        vwT = v_w[cols].T.astype(np.float16).reshape(8, 128, CW)
        im["vwb"] = np.ascontiguousarray(
            vwT.transpose(1, 0, 2).reshape(128, 8 * CW))
        im["owT"] = np.ascontiguousarray(out_w[:, cols].T, dtype=np.float16)
        in_maps.append(im)
    return in_maps


def kernel(x, w_q, b_q, w_k, b_k, v_w, out_w, _trace=False):
    x = np.asarray(x, dtype=np.float32)
    w_q = np.asarray(w_q, dtype=np.float32)
    b_q = np.asarray(b_q, dtype=np.float32)
    w_k = np.asarray(w_k, dtype=np.float32)
    b_k = np.asarray(b_k, dtype=np.float32)
    v_w = np.asarray(v_w, dtype=np.float32)
    out_w = np.asarray(out_w, dtype=np.float32)

    if "nc" not in _CACHE:
        _CACHE["nc"] = _build_nc()
    nc = _CACHE["nc"]

    in_maps = _prep_inputs(x, w_q, b_q, w_k, b_k, v_w, out_w)
    res = run_bass_kernel_spmd(
        nc, in_maps, core_ids=list(range(NCORES)), trace=_trace
    )
    out = np.zeros((B, S, D), dtype=np.float32)
    for c in range(NCORES):
        # outb{qw}[p, od*512+q] -> outT[od*128+p, qw*512+q] -> out (S, D)
        for qw in range(NQW):
            ob = res.results[c][f"outb{qw}"].astype(np.float32)
            ob = ob.reshape(128, 8, SQW).transpose(1, 0, 2).reshape(D, SQW)
            out[c // 4][qw * SQW:(qw + 1) * SQW] += ob.T
    if _trace:
        kernel._last_result = res
    return out


# revision 29
# speedup vs baseline: 1.0391x; 1.0391x over previous
"""Trainium2 Bass kernel for nn_EulerCausalAttention_75892072121064.

Sharding: batch*heads across 8 cores (core c -> batch c//4, heads 4*(c%4)..+4).
Each core computes transposed-layout causal attention for its (b, 4-head)
slice plus the out-proj partial, writing outT (D, S) in fp16; the host sums
the 4 per-batch partials and transposes back.

v2 design (vs the LUT-exact baseline):
- The reference's 4096-entry LUT quantizes theta to ~7.7e-4 rad; computing
  continuous sin/cos instead lands ~7e-4 rel err vs the 2e-2 gate, so the
  whole round/wrap DVE chain is replaced by host-side prep: the host ships
  wrapped angles (x*inv + b [+ pi/2 for cos rows], wrapped to [-pi, pi]) as
  fp16 [128, S] per (head, q/k), and the device runs ONE big Sin per tile
  (ACT cost is free-size only: 8 Sins of [128,2048] ~= 15us vs 64 of
  [64,512] ~= 46us).
- All matmul operands fp16 (full PE rate, 2x DVE ops, half the DMA).
  exp(score/sqrt(128) - 5) keeps attn weights in fp16 range (max score ~=
  125 on the q==k diagonal); the -5 cancels in normalization.
- Diagonal score blocks are column-trimmed: matmul/exp/attnV only touch the
  causally valid [r*128:512] window (saves ~12k PE columns + ~10us ACT).
- V-proj groups are interleaved with attention qw windows (group g feeds
  exactly the vt tiles window qw=g needs), so PE ramps while DMA streams.
- PSUM: scores+vproj share a 4-bank ring; attnV 2; outproj 2 = 8 banks.
"""
import sys

import numpy as np

sys.path.insert(0, "/opt/trn_rl_repo")

from concourse import bacc, mybir  # noqa: E402
import concourse.tile as tile  # noqa: E402
from concourse.bass_utils import run_bass_kernel_spmd  # noqa: E402

B, S, D, H, DH = 2, 2048, 1024, 16, 64
LUT = 4096
TWO_PI = 2.0 * np.pi
SCALE = float(np.sqrt(np.float32(2.0 * DH)))  # sqrt(128) as f32
NCORES = 8
HPC = 4            # heads per core
CW = HPC * DH      # 256 cols per core
SQW = 512          # q window
NQW = S // SQW
KBS = 128          # k block size
NS = S // 128      # seq tiles
EB = -5.0          # exp bias: keeps fp16 attn weights in range, cancels in norm

F32 = mybir.dt.float32
F16 = mybir.dt.float16
AF = mybir.ActivationFunctionType
ALU = mybir.AluOpType

_CACHE = {}


def _seed_const(nc, val):
    key = (F32, float(val))
    if key in nc.const_aps.aps:
        return
    t = nc.alloc_sbuf_tensor(f"const-f32-{val}", [128, 1], F32)
    nc.gpsimd.memset(t.ap(), float(val))
    nc.const_aps.aps[key] = t.ap()


def _build_nc():
    nc = bacc.Bacc("TRN2", debug=False, num_devices=NCORES)
    _seed_const(nc, 0.0)
    _seed_const(nc, EB)
    nc.all_engine_barrier()

    # thA = [q0|k0|q1|k1], thB = [q2|k2|q3|k3] along the free dim
    thA_d = nc.dram_tensor("thA", [128, 4 * S], F16, kind="ExternalInput")
    thB_d = nc.dram_tensor("thB", [128, 4 * S], F16, kind="ExternalInput")
    # x / vw shipped pre-swizzled so each lands in ONE DMA issue per tile:
    # xg_d[g][p, od*512+q] = x[b].T[od*128+p, g*512+q]
    xg_d = [nc.dram_tensor(f"xg{g}", [128, 8 * SQW], F16, kind="ExternalInput")
            for g in range(NQW)]
    vwb_d = nc.dram_tensor("vwb", [128, 8 * CW], F16, kind="ExternalInput")
    owT_d = nc.dram_tensor("owT", [CW, D], F16, kind="ExternalInput")
    tri_d = nc.dram_tensor("tri", [128, 128], F16, kind="ExternalInput")
    # outb[qw][p, od*512+q] = outT[od*128+p, qw*512+q]
    outb_d = [nc.dram_tensor(f"outb{qw}", [128, 8 * SQW], F16,
                             kind="ExternalOutput") for qw in range(NQW)]

    inv_scale = float(1.0 / np.float32(SCALE))

    with tile.TileContext(nc) as tc:
        with (
            tc.tile_pool(name="pp", bufs=1) as pp,
            tc.tile_pool(name="atp", bufs=2) as atp,
            tc.tile_pool(name="wk", bufs=2) as wk,
            tc.tile_pool(name="scp", bufs=2, space="PSUM") as scp,
            tc.tile_pool(name="opp", bufs=4, space="PSUM") as opp,
        ):
            # ---- DMA: vwb + thetas + x group 0 first, rest streamed ----
            vwb = pp.tile([128, 8 * CW], F16, tag="vwb")
            nc.sync.dma_start(vwb[:], vwb_d[:])
            xgb = [pp.tile([128, 8 * SQW], F16, tag=f"xg{g}", name=f"xg{g}")
                   for g in range(NQW)]
            qt, kt = [], []
            with tc.tile_pool(name="thp", bufs=2) as thp:
                th_src = [(thA_d, 0), (thA_d, 2 * S), (thB_d, 0),
                          (thB_d, 2 * S)]
                for h in range(HPC):
                    dram, off = th_src[h]
                    th = thp.tile([128, 2 * S], F16, tag="th", name=f"th{h}")
                    nc.sync.dma_start(th[:, 0:S], dram[:, off:off + S])
                    nc.sync.dma_start(th[:, S:2 * S],
                                      dram[:, off + S:off + 2 * S])
                    if h == 0:
                        nc.sync.dma_start(xgb[0][:], xg_d[0][:])
                    elif h == 1:
                        for g in range(1, NQW):
                            nc.sync.dma_start(xgb[g][:], xg_d[g][:])
                    elif h == 2:
                        tri_sb = pp.tile([128, 128], F16, tag="tri")
                        nc.sync.dma_start(tri_sb[:], tri_d[:])
                        ow2 = []
                        for hp in range(2):
                            t = pp.tile([128, D], F16, tag=f"ow{hp}",
                                        name=f"ow{hp}")
                            nc.sync.dma_start(
                                t[:], owT_d[hp * 128:(hp + 1) * 128, :])
                            ow2.append(t)
                    # one Sin per (head, q/k)
                    q = pp.tile([128, S], F16, tag=f"qt{h}", name=f"qt{h}")
                    nc.scalar.activation(q[:], th[:, 0:S], AF.Sin,
                                         bias=0.0, scale=1.0)
                    qt.append(q)
                    k = pp.tile([128, S], F16, tag=f"kt{h}", name=f"kt{h}")
                    nc.scalar.activation(k[:], th[:, S:2 * S], AF.Sin,
                                         bias=0.0, scale=1.0)
                    kt.append(k)

            vt = [pp.tile([128, HPC * 65], F16, tag=f"v{si}", name=f"v{si}")
                  for si in range(NS)]

            def vproj_group(g):
                for j in range(4):
                    si = 4 * g + j
                    vps = opp.tile([128, SQW], F32, tag="o", name="vps")
                    for od in range(8):
                        nc.tensor.matmul(
                            vps[:, 0:CW],
                            xgb[g][:, od * SQW + j * 128:
                                   od * SQW + (j + 1) * 128],
                            vwb[:, od * CW:(od + 1) * CW],
                            start=(od == 0), stop=(od == 7),
                        )
                    dst = vt[si][:].rearrange(
                        "p (h w) -> p h w", w=65)[:, :, 0:64]
                    src = vps[:, 0:CW].rearrange("p (h w) -> p h w", w=64)
                    nc.vector.tensor_copy(dst, src)
                    ones = vt[si][:].rearrange(
                        "p (h w) -> p h w", w=65)[:, :, 64:65]
                    nc.gpsimd.memset(ones, 1.0)

            vproj_group(0)
            vproj_group(1)

            # Attention, software-pipelined with a 2-head lag that CARRIES
            # ACROSS window boundaries: the attnV chain for a head whose exps
            # are long done is interleaved into the exp-paced score phase of
            # a later head (possibly of the next qw window), so the PE never
            # starves; norm/out-proj of window qw are emitted as soon as its
            # last attnV chain drains (mid window qw+1).
            states = {}
            pend = []          # (qw, h, kb) attnV work not yet emitted
            o_cur = {}         # (qw, h) -> psum tile of in-flight chain
            done_q = []        # qw values whose avs fully emitted

            def emit_avs(n):
                for _ in range(n):
                    if not pend:
                        return
                    q_, h2, kb = pend.pop(0)
                    st = states[q_]
                    nkb_ = st["nkb"]
                    if kb == 0:
                        o_cur[(q_, h2)] = opp.tile([128, SQW], F32, tag="o",
                                                   name="o_ps")
                    at, coff, c0 = st["atmap"][h2][kb]
                    nc.tensor.matmul(
                        o_cur[(q_, h2)][0:65, c0:SQW],
                        vt[kb][:, h2 * 65:(h2 + 1) * 65],
                        at[:, coff + c0:coff + SQW],
                        start=(kb == 0), stop=(kb == nkb_ - 1),
                        skip_group_check=True,
                    )
                    if kb == nkb_ - 1:
                        o_ps = o_cur.pop((q_, h2))
                        # ACT copy: proven partition-shift (psum p64 -> p0)
                        nc.scalar.copy(
                            st["srow4"][0:1, h2 * SQW:(h2 + 1) * SQW],
                            o_ps[64:65, :])
                        oraw = wk.tile([64, SQW], F16, tag=f"oraw{h2}",
                                       name=f"oraw{h2}", bufs=1)
                        nc.vector.tensor_copy(oraw[:], o_ps[0:64, :])
                        st["oraws"][h2] = oraw
                        if h2 == 3:
                            done_q.append(q_)

            def emit_norm_oproj():
                while done_q:
                    q_ = done_q.pop(0)
                    st = states.pop(q_)
                    srow4 = st["srow4"]
                    # in-place reciprocal of the 4 denominator rows
                    nc.vector.reciprocal_approx_fast(srow4[:], srow4[:])
                    sre16 = wk.tile([1, HPC * SQW], F16, tag="sre16",
                                    name="sre16")
                    nc.vector.tensor_copy(sre16[:], srow4[:])
                    pairs = [wk.tile([128, SQW], F16, tag=f"pair{hp}",
                                     name=f"pair{hp}") for hp in range(2)]
                    for h_ in range(HPC):
                        bc = wk.tile([64, SQW], F16, tag="bc", name="bc")
                        nc.gpsimd.partition_broadcast(
                            bc[:], sre16[0:1, h_ * SQW:(h_ + 1) * SQW])
                        rows = slice((h_ % 2) * 64, (h_ % 2) * 64 + 64)
                        nc.vector.tensor_tensor(pairs[h_ // 2][rows, :],
                                                st["oraws"][h_][:], bc[:],
                                                ALU.mult)
                    prq = wk.tile([128, 8 * SQW], F16, tag="prq", name="prq",
                                  bufs=1)
                    for od in range(8):
                        pr = opp.tile([128, SQW], F32, tag="o", name="pr")
                        nc.tensor.matmul(pr[:],
                                         ow2[0][:, od * 128:(od + 1) * 128],
                                         pairs[0][:], start=True, stop=False)
                        nc.tensor.matmul(pr[:],
                                         ow2[1][:, od * 128:(od + 1) * 128],
                                         pairs[1][:], start=False, stop=True)
                        nc.vector.tensor_copy(
                            prq[:, od * SQW:(od + 1) * SQW], pr[:])
                        if od == 3:
                            nc.sync.dma_start(outb_d[q_][:, 0:4 * SQW],
                                              prq[:, 0:4 * SQW])
                    nc.sync.dma_start(outb_d[q_][:, 4 * SQW:8 * SQW],
                                      prq[:, 4 * SQW:8 * SQW])

            for qw in range(NQW):
                nkb = 4 * qw + 4
                states[qw] = {
                    "nkb": nkb,
                    "srow4": wk.tile([1, HPC * SQW], F32, tag="srow4",
                                     name="srow4", bufs=2),
                    "atmap": [None] * HPC,
                    "oraws": [None] * HPC,
                }
                for h in range(HPC):
                    if h >= 2:
                        pend.extend((qw, h - 2, kb) for kb in range(nkb))
                    ats = {}
                    # off-diagonal kb pairs share one 2-bank psum supertile
                    # and ONE exp (consumers lag 2 heads behind)
                    for kb in range(0, 4 * qw, 2):
                        scs = scp.tile([128, 2 * SQW], F32, tag="sc",
                                       name="scs")
                        for half in range(2):
                            nc.tensor.matmul(
                                scs[:, half * SQW:(half + 1) * SQW],
                                kt[h][:, (kb + half) * KBS:
                                      (kb + half + 1) * KBS],
                                qt[h][:, qw * SQW:(qw + 1) * SQW],
                                start=True, stop=True,
                            )
                            emit_avs(1)
                            emit_norm_oproj()
                        at2 = atp.tile([128, 2 * SQW], F16, tag="at",
                                       name="at2", bufs=20)
                        nc.scalar.activation(at2[:], scs[:], AF.Exp,
                                             bias=EB, scale=inv_scale)
                        ats[kb] = (at2, 0, 0)
                        ats[kb + 1] = (at2, SQW, 0)
                    for r in range(4):
                        kb = 4 * qw + r
                        c0 = r * 128
                        scd = scp.tile([128, 2 * SQW], F32, tag="sc",
                                       name="scd")
                        nc.tensor.matmul(
                            scd[:, c0:SQW],
                            kt[h][:, kb * KBS:(kb + 1) * KBS],
                            qt[h][:, qw * SQW + c0:(qw + 1) * SQW],
                            start=True, stop=True,
                        )
                        atd = atp.tile([128, SQW], F16, tag="atd",
                                       name="atd", bufs=16)
                        nc.scalar.activation(atd[:, c0:SQW], scd[:, c0:SQW],
                                             AF.Exp, bias=EB, scale=inv_scale)
                        nc.vector.tensor_tensor(
                            atd[:, c0:c0 + 128], atd[:, c0:c0 + 128],
                            tri_sb[:], ALU.mult,
                        )
                        ats[kb] = (atd, 0, c0)
                        emit_avs(1)
                        emit_norm_oproj()
                    states[qw]["atmap"][h] = ats
                # drain the trailing 2 heads, then this window's norm/out-proj
                pend.extend((qw, 2, kb) for kb in range(nkb))
                pend.extend((qw, 3, kb) for kb in range(nkb))
                emit_avs(len(pend))

                # PE filler while the tail exps/norm drain
                if qw < 2:
                    vproj_group(qw + 2)
                emit_norm_oproj()

    nc.compile()
    return nc


def _prep_inputs(x, w_q, b_q, w_k, b_k, v_w, out_w):
    """Build the 8 per-core input maps (host-side sharding + angle prep)."""
    in_maps = []
    tri = np.triu(np.ones((128, 128), dtype=np.float16))  # valid: q >= k
    half_pi = np.pi / 2

    def wrap(v):
        return ((v + np.pi) % TWO_PI) - np.pi

    for c in range(NCORES):
        b = c // 4
        h0 = (c % 4) * HPC
        cols = np.arange(h0 * DH, h0 * DH + CW)
        im = {"tri": tri}
        xb = x[b].astype(np.float64)
        ths = []
        for hh in range(HPC):
            hg = h0 + hh
            for nm, w_, b_ in (("q", w_q, b_q), ("k", w_k, b_k)):
                th = xb[:, hg * DH:(hg + 1) * DH] / (
                    1.0 + np.abs(w_[hg].astype(np.float64))) + b_[hg]
                t = np.concatenate([wrap(th + half_pi).T, wrap(th).T], axis=0)
                ths.append(t.astype(np.float16))
        im["thA"] = np.ascontiguousarray(np.concatenate(ths[0:4], axis=1))
        im["thB"] = np.ascontiguousarray(np.concatenate(ths[4:8], axis=1))
        # xg{g}[p, od*512+q] = x[b].T[od*128+p, g*512+q]
        xt = x[b].T.astype(np.float16).reshape(8, 128, NQW, SQW)
        for g in range(NQW):
            im[f"xg{g}"] = np.ascontiguousarray(
                xt[:, :, g, :].transpose(1, 0, 2).reshape(128, 8 * SQW))
        # vwb[p, od*256+j] = v_w[cols].T[od*128+p, j]
        vwT = v_w[cols].T.astype(np.float16).reshape(8, 128, CW)
        im["vwb"] = np.ascontiguousarray(
            vwT.transpose(1, 0, 2).reshape(128, 8 * CW))
        im["owT"] = np.ascontiguousarray(out_w[:, cols].T, dtype=np.float16)
        in_maps.append(im)
    return in_maps


def kernel(x, w_q, b_q, w_k, b_k, v_w, out_w, _trace=False):
    x = np.asarray(x, dtype=np.float32)
    w_q = np.asarray(w_q, dtype=np.float32)
    b_q = np.asarray(b_q, dtype=np.float32)
    w_k = np.asarray(w_k, dtype=np.float32)
    b_k = np.asarray(b_k, dtype=np.float32)
    v_w = np.asarray(v_w, dtype=np.float32)
    out_w = np.asarray(out_w, dtype=np.float32)

    if "nc" not in _CACHE:
        _CACHE["nc"] = _build_nc()
    nc = _CACHE["nc"]

    in_maps = _prep_inputs(x, w_q, b_q, w_k, b_k, v_w, out_w)
    res = run_bass_kernel_spmd(
        nc, in_maps, core_ids=list(range(NCORES)), trace=_trace
    )
    out = np.zeros((B, S, D), dtype=np.float32)
    for c in range(NCORES):
        # outb{qw}[p, od*512+q] -> outT[od*128+p, qw*512+q] -> out (S, D)
        for qw in range(NQW):
            ob = res.results[c][f"outb{qw}"].astype(np.float32)
            ob = ob.reshape(128, 8, SQW).transpose(1, 0, 2).reshape(D, SQW)
            out[c // 4][qw * SQW:(qw + 1) * SQW] += ob.T
    if _trace:
        kernel._last_result = res
    return out
